# revision 1
# baseline (speedup 1.0000x reference)
"""Masked multi-head attention (B=32, Lq=Lk=512, H=20, D=20) on 8 TRN2 NeuronCores.

Strategy:
  - Data-parallel over batch: 32 batches -> 8 cores x 4 "slots" (SPMD: one NEFF).
  - Host bakes per-slot static shapes (nq = padded Q_len, nkc = kv chunks from
    V_len) and bin-packs batches into slot groups to minimize padded work.
  - Host pre-transposes sequences to [21, L] (20 features + ones row).  The
    ones row realizes: exact linear bias, zeroing of masked kv positions
    (mask folded into V/K inputs), and a free softmax-denominator column in
    the projected V tile.
  - Device per (slot, head-group of 4 heads at 32-partition offsets):
      proj Q/K/V (PE, contraction 21)
      S^T = K_h @ Q_h^T   row-tiled 4 heads concurrently  (PSUM)
      P^T = exp(S^T / sqrt(D))  one ACTIVATE per pack     (SBUF)
      O^T(+sums row) = [V_h|mask]^T @ P^T  col-tiled, accumulated over kv chunks
      PE transpose -> [q, .] layout; DVE reciprocal + broadcast multiply
      assemble [128, 400] and DMA to DRAM.
  - Host scatters per-slot outputs into the final [32, 512, 400] (rows beyond
    Q_len stay zero, which implements the multiplicative q mask exactly).
"""

import math
import random

import numpy as np

import concourse.bacc as bacc
import concourse.bass as bass
import concourse.tile as tile
from concourse import mybir
from concourse.bass_utils import run_bass_kernel_spmd

B, LQ, LK = 32, 512, 512
H, D = 20, 20
OUT_DIM = H * D  # 400
N_CORES = 8
N_SLOTS = B // N_CORES  # 4
QCH = 128
KCH = 128
NG = 5  # head groups
HPG = 4  # heads per group (at partition offsets 0/32/64/96)
VW = H * 21 + 12  # 432 (even, for fp32r): per-head 20 dims + 1 ones col,
                  # padded so a 32-wide lhsT slice exists for every head
SCALE = 1.0 / math.sqrt(D)
# Constant shift inside exp: P = exp(s/sqrt(D) - ESHIFT).  Softmax is
# shift-invariant (sums scale by e^-ESHIFT), and the shift keeps P below
# fp16 max (65504) for scores up to ~17 sigma.  Zero-flush of the tiniest
# weights (P < 6e-8) is harmless: they are >= e^9 below their column max.
ESHIFT = 6.0

F32 = mybir.dt.float32

# Perf knobs
USE_F32R = True  # bitcast matmul operands to float32r (fast fp32 path)
TRACE = False  # set True to capture NTFF profile (slower)
LAST_RESULT = None  # BassKernelResults of the last run (for test harness)


# ----------------------------------------------------------------- planning

def _plan(q_len, v_len):
    """Group 32 batches into N_SLOTS groups of N_CORES, minimizing baked cost.

    Returns list of (nq, nkc, batches[8]) sorted big->small."""
    nqc = [max(1, math.ceil(min(int(q), LQ) / QCH)) for q in q_len]
    kv_eff = [LK if int(v) <= 0 else min(int(v), LK) for v in v_len]
    nkc = [math.ceil(k / KCH) for k in kv_eff]
    cost = [a * b for a, b in zip(nqc, nkc)]
    order = sorted(range(B), key=lambda b: -cost[b])

    def baked(gs):
        t = 0
        for g in gs:
            if g:
                t += max(nqc[b] for b in g) * max(nkc[b] for b in g)
        return t

    groups = [[] for _ in range(N_SLOTS)]
    for b in order:
        best, bestc = None, None
        for gi in range(N_SLOTS):
            if len(groups[gi]) >= N_CORES:
                continue
            groups[gi].append(b)
            c = baked(groups)
            groups[gi].pop()
            if bestc is None or c < bestc:
                best, bestc = gi, c
        groups[best].append(b)
    rng = random.Random(0)
    cur = baked(groups)
    for _ in range(6000):
        g1, g2 = rng.randrange(N_SLOTS), rng.randrange(N_SLOTS)
        if g1 == g2:
            continue
        i1, i2 = rng.randrange(N_CORES), rng.randrange(N_CORES)
        groups[g1][i1], groups[g2][i2] = groups[g2][i2], groups[g1][i1]
        c = baked(groups)
        if c <= cur:
            cur = c
        else:
            groups[g1][i1], groups[g2][i2] = groups[g2][i2], groups[g1][i1]
    slots = []
    for g in groups:
        snq = max(nqc[b] for b in g) * QCH
        snkc = max(nkc[b] for b in g)
        slots.append((snq, snkc, list(g)))
    slots.sort(key=lambda s: -(s[0] * s[1]))
    return slots


# ------------------------------------------------------------ host packing

def _pack_qk_weights(W, bias):
    """[400, 20] linear weight -> [21, NG*128] lhsT layout (head 4g+j at
    columns 128g+32j .. +20; row 20 = bias)."""
    t = np.zeros((D + 1, NG * 128), np.float32)
    for h in range(H):
        g, j = divmod(h, HPG)
        c = g * 128 + 32 * j
        t[:D, c:c + D] = W[h * D:(h + 1) * D, :].T
        t[D, c:c + D] = bias[h * D:(h + 1) * D]
    return t


def _pack_v_weights(W, bias):
    """[400, 20] -> [21, 420] rhs layout: head h at cols 21h..21h+19,
    ones-generator col at 21h+20."""
    t = np.zeros((D + 1, VW), np.float32)
    for h in range(H):
        c = 21 * h
        t[:D, c:c + D] = W[h * D:(h + 1) * D, :].T
        t[D, c:c + D] = bias[h * D:(h + 1) * D]
        t[D, c + D] = 1.0
    return t


def _prep_qt(qs, nq):
    t = np.zeros((D + 1, nq), np.float32)
    n = min(nq, LQ)
    t[:D, :n] = qs[:n].T
    t[D, :n] = 1.0
    return t


def _prep_kvt(ks, vlen, nkv):
    """K/V sequence transposed with ones row; columns >= V_len zeroed
    (vlen==0 means "uniform -1e12 shift" in the reference == full attention)."""
    t = np.zeros((D + 1, nkv), np.float32)
    n = min(nkv, LK) if int(vlen) <= 0 else min(nkv, int(vlen))
    t[:D, :n] = ks[:n].T
    t[D, :n] = 1.0
    return t


# ------------------------------------------------------------ device build

def _emit(tc, nc, dr, slots):
    # fp32r matmul operands must come from instructions that round to fp32r;
    # DMA can't, so DMA'd tensors get one DVE rounding copy each.
    DT = mybir.dt.float32r if USE_F32R else F32
    with (
        tc.tile_pool(name="wpool", bufs=1) as wpool,
        tc.tile_pool(name="seqin", bufs=2) as seqp,
        tc.tile_pool(name="sbq", bufs=3) as sbqp,
        tc.tile_pool(name="sbk", bufs=3) as sbkp,
        tc.tile_pool(name="sbv", bufs=6) as sbvp,
        tc.tile_pool(name="sbp", bufs=4) as sbpp,
        tc.tile_pool(name="sbo", bufs=2) as sbop,
        tc.tile_pool(name="sbr", bufs=4) as sbrp,
        tc.tile_pool(name="asm", bufs=6) as asmp,
        tc.tile_pool(name="ppj", bufs=1, space="PSUM") as ppj,
        tc.tile_pool(name="pss", bufs=2, space="PSUM") as pss,
        tc.tile_pool(name="pso", bufs=2, space="PSUM") as pso,
        tc.tile_pool(name="pst", bufs=1, space="PSUM") as pst,
    ):
        def load_rounded(name, shape, pool, tag):
            raw = pool.tile(shape, F32, tag=tag + "_raw", name=name + "_raw")
            nc.sync.dma_start(raw[:], dr[name])
            if not USE_F32R:
                return raw
            t = pool.tile(shape, DT, tag=tag, name=name + "_r")
            nc.vector.tensor_copy(t[:], raw[:])
            return t

        wq = load_rounded("wq", [D + 1, NG * 128], wpool, "wq")
        wk = load_rounded("wk", [D + 1, NG * 128], wpool, "wk")
        wv = load_rounded("wv", [D + 1, VW], wpool, "wv")
        ident = load_rounded("ident", [128, 128], wpool, "ident")
        eshift = wpool.tile([128, 1], F32, tag="eshift")
        nc.vector.memset(eshift[:], -ESHIFT)

        for s, (nq, nkc, _g) in enumerate(slots):
            nkv = nkc * KCH
            nqc = nq // QCH
            # 2 heads per S^T psum tile; each head's [128, nq] slice padded to a
            # full 2KB bank so no two matmul outputs share a PSUM zero region.
            hp = 2

            qt = load_rounded(f"qt{s}", [D + 1, nq], seqp, "qt")
            kt = load_rounded(f"kt{s}", [D + 1, nkv], seqp, "kt")
            vt = load_rounded(f"vt{s}", [D + 1, nkv], seqp, "vt")

            # V projection: per kv chunk -> [128, 420] (incl. masked ones cols)
            sbV = []
            for kc in range(nkc):
                pv = ppj.tile([128, 512], F32, tag="ppj")
                nc.tensor.matmul(
                    pv[:, :VW], vt[:, kc * KCH:(kc + 1) * KCH], wv[:],
                    start=True, stop=True,
                )
                v = sbvp.tile([128, VW], mybir.dt.float16, tag="sbv")
                nc.vector.tensor_copy(v[:], pv[:, :VW])
                sbV.append(v)

            asms = [
                asmp.tile([128, OUT_DIM], F32, tag="asm", name=f"asm{s}_{qc}")
                for qc in range(nqc)
            ]

            for g in range(NG):
                pq = ppj.tile([128, 512], F32, tag="ppj")
                nc.tensor.matmul(
                    pq[:, :nq], wq[:, g * 128:(g + 1) * 128], qt[:],
                    start=True, stop=True,
                )
                q = sbqp.tile([128, nq], mybir.dt.float16, tag="sbq")
                nc.vector.tensor_copy(q[:], pq[:, :nq])

                pk = ppj.tile([128, 512], F32, tag="ppj")
                nc.tensor.matmul(
                    pk[:, :nkv], wk[:, g * 128:(g + 1) * 128], kt[:],
                    start=True, stop=True,
                )
                k = sbkp.tile([128, nkv], mybir.dt.float16, tag="sbk")
                nc.vector.tensor_copy(k[:], pk[:, :nkv])

                po = pso.tile([128, nq], F32, tag="pso")

                for kc in range(nkc):
                    # all 4 S^T matmuls back-to-back (distinct row groups ->
                    # they pipeline/overlap in the PE's 32x32 subarrays),
                    # then the exps, then the 4 O^T matmuls (distinct col
                    # groups).  Interleaving full-row-span work between
                    # row-tiled matmuls would serialize the subarrays.
                    packs = []
                    for jp in range(0, HPG, hp):
                        ps = pss.tile([128, hp, 512], F32, tag="pss",
                                      name=f"ps{s}_{g}_{kc}_{jp}")
                        for j in range(jp, jp + hp):
                            nc.tensor.matmul(
                                ps[:, j - jp, :nq],
                                k[32 * j:32 * j + D, kc * KCH:(kc + 1) * KCH],
                                q[32 * j:32 * j + D, :],
                                start=True, stop=True,
                                tile_position=(32 * j, 0),
                            )
                        packs.append(ps)
                    ptiles = []
                    for jp, ps in zip(range(0, HPG, hp), packs):
                        p = sbpp.tile([128, hp, 512], mybir.dt.float16,
                                      tag="sbp", name=f"p{s}_{g}_{kc}_{jp}")
                        nc.scalar.activation(
                            p[:, :, :nq], ps[:, :, :nq],
                            mybir.ActivationFunctionType.Exp,
                            bias=eshift[:], scale=SCALE,
                        )
                        ptiles.append(p)
                    for jp, p in zip(range(0, HPG, hp), ptiles):
                        for j in range(jp, jp + hp):
                            h = HPG * g + j
                            # col-tiled accumulation chains touch disjoint
                            # partition ranges (32j..32j+20) of one bank; the
                            # sim's zero-region check is bank-granular, so
                            # bypass it.
                            nc.tensor.matmul(
                                po[32 * j:32 * j + 32, :],
                                sbV[kc][:, 21 * h:21 * h + 32],
                                p[:, j - jp, :nq],
                                start=(kc == 0), stop=(kc == nkc - 1),
                                tile_position=(0, 32 * j),
                                skip_group_check=True,
                            )

                o = sbop.tile([128, nq], DT, tag="sbo")
                nc.vector.tensor_copy(o[:], po[:])
                for qc in range(nqc):
                    pt = pst.tile([128, 128], DT, tag="pst")
                    nc.tensor.transpose(pt[:], o[:, qc * QCH:(qc + 1) * QCH], ident[:])
                    # f32r bits are valid f32; read back as f32 for DVE ops
                    ptb = pt.bitcast(F32).rearrange("p (j c) -> p j c", j=HPG)
                    r = sbrp.tile([128, HPG], F32, tag="sbr")
                    nc.vector.reciprocal(r[:], ptb[:, :, D])
                    nc.vector.tensor_mul(
                        asms[qc][:, g * 80:(g + 1) * 80]
                            .rearrange("p (j d) -> p j d", j=HPG),
                        ptb[:, :, 0:D],
                        r.unsqueeze(2).broadcast_to([128, HPG, D]),
                    )

            for qc in range(nqc):
                nc.sync.dma_start(
                    dr[f"o{s}"][qc * QCH:(qc + 1) * QCH, :], asms[qc][:]
                )


def _build_nc(slots):
    nc = bacc.Bacc(
        "TRN2",
        target_bir_lowering=False,
        debug=False,
        enable_asserts=False,
        num_devices=N_CORES,
    )
    dr = {}
    for s, (nq, nkc, _grp) in enumerate(slots):
        nkv = nkc * KCH
        dr[f"qt{s}"] = nc.dram_tensor(f"qt{s}", [D + 1, nq], F32, kind="ExternalInput").ap()
        dr[f"kt{s}"] = nc.dram_tensor(f"kt{s}", [D + 1, nkv], F32, kind="ExternalInput").ap()
        dr[f"vt{s}"] = nc.dram_tensor(f"vt{s}", [D + 1, nkv], F32, kind="ExternalInput").ap()
        dr[f"o{s}"] = nc.dram_tensor(f"o{s}", [nq, OUT_DIM], F32, kind="ExternalOutput").ap()
    dr["wq"] = nc.dram_tensor("wq", [D + 1, NG * 128], F32, kind="ExternalInput").ap()
    dr["wk"] = nc.dram_tensor("wk", [D + 1, NG * 128], F32, kind="ExternalInput").ap()
    dr["wv"] = nc.dram_tensor("wv", [D + 1, VW], F32, kind="ExternalInput").ap()
    dr["ident"] = nc.dram_tensor("ident", [128, 128], F32, kind="ExternalInput").ap()

    with tile.TileContext(nc) as tc:
        _emit(tc, nc, dr, slots)
    nc.compile()
    return nc


# ----------------------------------------------------------------- driver

def kernel(**inputs):
    global LAST_RESULT
    Q_seq = np.ascontiguousarray(np.asarray(inputs["Q_seq"], dtype=np.float32))
    K_seq = np.ascontiguousarray(np.asarray(inputs["K_seq"], dtype=np.float32))
    V_seq = np.ascontiguousarray(np.asarray(inputs["V_seq"], dtype=np.float32))
    Q_len = np.asarray(inputs["Q_len"]).reshape(-1).astype(np.int64)
    V_len = np.asarray(inputs["V_len"]).reshape(-1).astype(np.int64)
    WQ_w = np.asarray(inputs["WQ_w"], dtype=np.float32)
    WQ_b = np.asarray(inputs["WQ_b"], dtype=np.float32)
    WK_w = np.asarray(inputs["WK_w"], dtype=np.float32)
    WK_b = np.asarray(inputs["WK_b"], dtype=np.float32)
    WV_w = np.asarray(inputs["WV_w"], dtype=np.float32)
    WV_b = np.asarray(inputs["WV_b"], dtype=np.float32)

    slots = _plan(Q_len, V_len)
    nc = _build_nc(slots)

    wq = _pack_qk_weights(WQ_w, WQ_b)
    wk = _pack_qk_weights(WK_w, WK_b)
    wv = _pack_v_weights(WV_w, WV_b)
    ident = np.eye(128, dtype=np.float32)

    in_maps = []
    for c in range(N_CORES):
        m = {"wq": wq, "wk": wk, "wv": wv, "ident": ident}
        for s, (nq, nkc, grp) in enumerate(slots):
            b = grp[c]
            nkv = nkc * KCH
            m[f"qt{s}"] = _prep_qt(Q_seq[b], nq)
            m[f"kt{s}"] = _prep_kvt(K_seq[b], V_len[b], nkv)
            m[f"vt{s}"] = _prep_kvt(V_seq[b], V_len[b], nkv)
        in_maps.append(m)

    res = run_bass_kernel_spmd(
        nc, in_maps, core_ids=list(range(N_CORES)), trace=TRACE
    )
    LAST_RESULT = res

    out = np.zeros((B, LQ, OUT_DIM), np.float32)
    for c in range(N_CORES):
        for s, (_nq, _nkc, grp) in enumerate(slots):
            b = grp[c]
            ql = int(Q_len[b])
            if ql > 0:
                out[b, :ql] = res.results[c][f"o{s}"][:ql]
    return out



# revision 2
# speedup vs baseline: 1.0083x; 1.0083x over previous
"""Masked MHA (B=32, Lq=Lk=512, H=20, D=20) on 8 TRN2 NeuronCores — v2.

Decomposition: units = (batch, q-chunk<=128) -> ~11 SPMD slots/core, slot
shapes (q_s, kv_s) = max over the 8 cores' units (exact, not 128-padded).

Score fold: S_h = q' A_h k'^T with A_h = [WQ_h|bq_h]^T [WK_h|bk_h] (21x21,
host-precomputed); q'/k' ones-augmented raw sequences. The device never
projects K: S^T matmul streams q2 = A^T q'^T against raw k' as lhsT.

Per (slot, kv-chunk): 5 rounds of row-tiled 4-packs write S^T for all 20
heads into one 5-bank PSUM region laid out [128, 4(j), 5(g), 128] so each
round's 4 concurrent tiles land in 4 distinct banks (offset (5j+g)*512B).
One ACTIVATE (exp, N=20*q_s) evacuates it to fp16; col-tiled O^T 4-packs
accumulate [V|1]^T P^T into a 2-bank po region across kv chunks.  The
fifth+ PSUM bank rotates projection singles (q2, V-hat).  Output = po
(numerators + denominator rows) shipped fp16; host transposes, divides,
scatters (rows beyond Q_len stay zero = multiplicative q mask).
"""

import math

import numpy as np

import concourse.bacc as bacc
import concourse.bass as bass
import concourse.tile as tile
from concourse import mybir
from concourse.bass_utils import run_bass_kernel_spmd

B, LQ, LK = 32, 512, 512
H, D = 20, 20
OUT_DIM = H * D
N_CORES = 8
QCH = 102
KCH = 128
SCALE = 1.0 / math.sqrt(D)
ESHIFT = 6.0
VW = 432

F32 = mybir.dt.float32
F16 = mybir.dt.float16

TRACE = False
LAST_RESULT = None


# ----------------------------------------------------------------- planning

def _lengths(q_len, v_len):
    qs, ks = [], []
    for b in range(B):
        q = max(0, min(int(q_len[b]), LQ))
        v = int(v_len[b])
        k = LK if v <= 0 else min(v, LK)
        qs.append(q)
        ks.append(k)
    return qs, ks


def _plan(q_len, v_len):
    """Units (batch, q0, q_e, kvlen) -> grid[slot][core]; slot shapes baked
    as max over the row. Local search minimizes estimated ACT time."""
    qs, ks = _lengths(q_len, v_len)
    units = []
    for b in range(B):
        for q0 in range(0, qs[b], QCH):
            units.append((b, q0, min(QCH, qs[b] - q0), ks[b]))
    units.sort(key=lambda u: (-math.ceil(u[3] / KCH), -u[2], -u[3]))
    n_slots = max(1, math.ceil(len(units) / N_CORES))
    grid = [[None] * N_CORES for _ in range(n_slots)]
    for i, u in enumerate(units):
        grid[i // N_CORES][i % N_CORES] = u

    def cost(g):
        t = 0.0
        for row in g:
            real = [u for u in row if u]
            if not real:
                continue
            q_s = max(u[2] for u in real)
            nkc = math.ceil(max(u[3] for u in real) / KCH)
            t += nkc * (20 * q_s + 590)
        return t

    import random
    rng = random.Random(0)
    cur = cost(grid)
    best = cur
    best_grid = [row[:] for row in grid]
    for it in range(120000):
        s1, s2 = rng.randrange(n_slots), rng.randrange(n_slots)
        if s1 == s2:
            continue
        c1, c2 = rng.randrange(N_CORES), rng.randrange(N_CORES)
        grid[s1][c1], grid[s2][c2] = grid[s2][c2], grid[s1][c1]
        c = cost(grid)
        if c <= cur or (it % 89 == 0 and c < cur * 1.015):
            cur = c
            if c < best:
                best = c
                best_grid = [row[:] for row in grid]
        else:
            grid[s1][c1], grid[s2][c2] = grid[s2][c2], grid[s1][c1]
    grid = best_grid
    slots = []
    for row in grid:
        real = [u for u in row if u]
        q_s = max(u[2] for u in real) if real else 1
        kv_s = max(u[3] for u in real) if real else 1
        slots.append((q_s, kv_s, math.ceil(kv_s / KCH)))
    order = sorted(range(n_slots), key=lambda s: -(slots[s][0] * slots[s][2]))
    return [slots[s] for s in order], [grid[s] for s in order]


# ------------------------------------------------------------ host packing

def _pack_a(WQ, bq, WK, bk):
    """apack [64, 5*128] fp16: rows 32r+i (2 replicas), col 128g+32j+c =
    A_{4g+j}[i,c] where A_h = [WQ_h|bq_h]^T [WK_h|bk_h]."""
    t = np.zeros((64, 5 * 128), np.float32)
    for h in range(H):
        WQa = np.concatenate([WQ[h*D:(h+1)*D, :], bq[h*D:(h+1)*D, None]], 1)
        WKa = np.concatenate([WK[h*D:(h+1)*D, :], bk[h*D:(h+1)*D, None]], 1)
        A = WQa.T @ WKa
        g, j = divmod(h, 4)
        for r in range(2):
            t[32*r:32*r+21, 128*g+32*j:128*g+32*j+21] = A
    return t.astype(np.float16)


def _pack_wv(WV, bv):
    """wvp [64, 432] fp16 (2 replicas at 32-offsets): [32r+i, 21h+d] =
    WV_h[d, i]; row 20 = bias; ones col at 21h+20."""
    t = np.zeros((64, VW), np.float32)
    for h in range(H):
        c = 21 * h
        for r in range(2):
            t[32*r:32*r+D, c:c+D] = WV[h*D:(h+1)*D, :].T
            t[32*r+D, c:c+D] = bv[h*D:(h+1)*D]
            t[32*r+D, c+D] = 1.0
    return t.astype(np.float16)


def _prep_rep(seq, n, nvalid, reps):
    """[L, 20] -> [32*reps, n] fp16: `reps` replicas of (20 dims + ones row)
    at 32-partition offsets; cols >= nvalid zero."""
    t = np.zeros((32 * reps, n), np.float32)
    m = min(nvalid, n)
    for a in range(reps):
        r = 32 * a
        t[r:r+D, :m] = seq[:m].T
        t[r+D, :m] = 1.0
    return t.astype(np.float16)


# ------------------------------------------------------------ device build

def _emit(tc, nc, dr, slots):
    n_slots = len(slots)
    with (
        tc.tile_pool(name="wts", bufs=1) as wts,
        tc.tile_pool(name="seq", bufs=2) as seqp,
        tc.tile_pool(name="q2s", bufs=12) as q2p,
        tc.tile_pool(name="vs", bufs=10) as vsp,
        tc.tile_pool(name="px16", bufs=2) as pxp,
        tc.tile_pool(name="ot", bufs=2) as otp,
        tc.tile_pool(name="pxa", bufs=1, space="PSUM") as pxa,
        tc.tile_pool(name="pxb", bufs=1, space="PSUM") as pxb,
        tc.tile_pool(name="pso", bufs=1, space="PSUM") as pso,
        tc.tile_pool(name="psj", bufs=1, space="PSUM") as psj,
    ):
        apack = wts.tile([64, 5 * 128], F16, tag="apack")
        nc.sync.dma_start(apack[:], dr["apack"])
        wvp = wts.tile([64, VW], F16, tag="wvp")
        nc.sync.dma_start(wvp[:], dr["wvp"])
        esh = wts.tile([128, 1], F32, tag="esh")
        nc.vector.memset(esh[:], -ESHIFT)

        q2s = {}   # (s, g) -> sbuf tile [128, q_s]
        vts = {}   # (s, kc) -> sbuf tile [128, VW]

        def proj_ops(s):
            """Return list of closures, each one proj single (mm + evac)."""
            q_s, kv_s, nkc = slots[s]
            qt = seqp.tile([64, q_s], F16, tag="qt", name=f"qt{s}")
            nc.sync.dma_start(qt[:], dr[f"qt{s}"])
            vt = seqp.tile([64, kv_s], F16, tag="vt", name=f"vt{s}")
            nc.sync.dma_start(vt[:], dr[f"vt{s}"])
            kt = seqp.tile([128, kv_s], F16, tag="kt", name=f"kt{s}")
            nc.sync.dma_start(kt[:], dr[f"kt{s}"])
            ops = []

            q2cat = q2p.tile([128, 5 * q_s], F16, tag="q2", name=f"q2_{s}")
            q2s[s] = q2cat

            def mk_q2(m):
                def op():
                    gs = [g for g in (2*m, 2*m+1) if g < 5]
                    pj = psj.tile([128, 2, 512], F32, tag="pj",
                                  name=f"pjq{s}_{m}")
                    for j2, g in enumerate(gs):
                        nc.tensor.matmul(
                            pj[:, j2, :q_s],
                            apack[32*j2:32*j2+21, 128*g:128*(g+1)],
                            qt[32*j2:32*j2+21, :], start=True, stop=True,
                            tile_position=(32*j2, 0),
                            skip_group_check=True,
                        )
                    if len(gs) == 2:
                        nc.vector.tensor_copy(
                            q2cat[:, 2*m*q_s:(2*m+2)*q_s]
                            .rearrange("p (a b) -> p a b", a=2),
                            pj[:, :, :q_s])
                    else:
                        nc.vector.tensor_copy(q2cat[:, 4*q_s:5*q_s],
                                              pj[:, 0, :q_s])
                return op

            def mk_v(m):
                def op():
                    kcs = [kc for kc in (2*m, 2*m+1) if kc < nkc]
                    pj = psj.tile([128, 2, 512], F32, tag="pj",
                                  name=f"pjv{s}_{m}")
                    t = vsp.tile([128, 2, VW], F16, tag="v", name=f"v{s}_{m}")
                    for a, kc in enumerate(kcs):
                        kv_c = min(KCH, kv_s - kc * KCH)
                        nc.tensor.matmul(
                            pj[:kv_c, a, :VW],
                            vt[32*a:32*a+21, kc*KCH:kc*KCH + kv_c],
                            wvp[32*a:32*a+21, :], start=True, stop=True,
                            tile_position=(32*a, 0),
                            skip_group_check=True,
                        )
                        nc.vector.tensor_copy(t[:kv_c, a, :], pj[:kv_c, a, :VW])
                        vts[(s, kc)] = (t, a)
                return op

            for m in range(3):
                ops.append(mk_q2(m))
            for m in range(math.ceil(nkc / 2)):
                ops.append(mk_v(m))
            return (kt, ops)

        kts = {}
        kts[0], pend = proj_ops(0)
        for op in pend:
            op()
        pend = []
        pending_tail = [None]  # deferred last-O^T + evac of previous slot

        def mk_emit_ot(s, q_s, kv_s, nkc, poAll):
            def emit_ot(p16, kc):
                p16a, p16b = p16
                kv_c = min(KCH, kv_s - kc * KCH)
                for g in range(5):
                    bank, half = g % 2, g // 2
                    for j in range(4):
                        h = 4 * g + j
                        dst = poAll[32*j:32*j+32, bank,
                                    128*half:128*half + q_s]
                        rhs = (p16a[:kv_c, g*q_s:(g+1)*q_s] if j == 0
                               else p16b[:kv_c, j - 1, g*q_s:(g+1)*q_s])
                        vt_t, vt_a = vts[(s, kc)]
                        mm = nc.tensor.matmul(
                            dst,
                            vt_t[:kv_c, vt_a, 21*h:21*h+32],
                            rhs,
                            start=(kc == 0 and half == 0),
                            stop=(kc == nkc - 1),
                            tile_position=(0, 32 * j),
                            skip_group_check=True,
                        )
                        tc.chain_iter_dep(f"po_b{bank}_{j}", mm.ins)
            return emit_ot

        def mk_tail(s, q_s, poAll, emit_ot, prev):
            def tail():
                emit_ot(*prev)
                ot = otp.tile([128, 5, q_s], F16, tag="ot", name=f"ot{s}")
                nc.vector.tensor_copy(ot[:, 0:2, :], poAll[:, :, 0:q_s])
                nc.vector.tensor_copy(ot[:, 2:4, :],
                                      poAll[:, :, 128:128 + q_s])
                nc.vector.tensor_copy(ot[:, 4, :],
                                      poAll[:, 0, 256:256 + q_s])
                nc.sync.dma_start(dr[f"ot{s}"], ot[:])
            return tail

        for s, (q_s, kv_s, nkc) in enumerate(slots):
            kt = kts[s]
            if s + 1 < n_slots:
                kts[s + 1], pend = proj_ops(s + 1)
            share = math.ceil(len(pend) / nkc) if pend else 0

            poAll = None
            emit_ot = None
            prev = None  # (px16, kc) awaiting O^T

            for kc in range(nkc):
                kv_c = min(KCH, kv_s - kc * KCH)
                qa = 5 * q_s
                pa = pxa.tile([128, 512], F32, tag="pxa", name=f"pa{s}_{kc}")
                pb = pxb.tile([128, 3, 512], F32, tag="pxb",
                              name=f"pb{s}_{kc}")
                nc.tensor.matmul(
                    pa[:kv_c, :qa],
                    kt[0:21, kc*KCH:kc*KCH + kv_c],
                    q2s[s][0:21, :],
                    start=True, stop=True, tile_position=(0, 0),
                    skip_group_check=True,
                )
                for j in range(1, 4):
                    nc.tensor.matmul(
                        pb[:kv_c, j - 1, :qa],
                        kt[32*j:32*j+21, kc*KCH:kc*KCH + kv_c],
                        q2s[s][32*j:32*j+21, :],
                        start=True, stop=True, tile_position=(32 * j, 0),
                        skip_group_check=True,
                    )
                # next slot's projections ride in the ACT shadow
                for _ in range(share):
                    if pend:
                        pend.pop(0)()
                if kc == 0:
                    if pending_tail[0] is not None:
                        pending_tail[0]()
                    poAll = pso.tile([128, 2, 512], F32, tag="po",
                                     name=f"po{s}")
                    emit_ot = mk_emit_ot(s, q_s, kv_s, nkc, poAll)
                else:
                    emit_ot(*prev)
                p16a = pxp.tile([128, 5 * q_s], F16, tag="p16a",
                                name=f"p16a{s}_{kc}")
                p16b = pxp.tile([128, 3, 5 * q_s], F16, tag="p16b",
                                name=f"p16b{s}_{kc}")
                nc.scalar.activation(
                    p16a[:kv_c], pa[:kv_c, :qa],
                    mybir.ActivationFunctionType.Exp,
                    bias=esh[:kv_c], scale=SCALE,
                )
                nc.scalar.activation(
                    p16b[:kv_c], pb[:kv_c, :, :qa],
                    mybir.ActivationFunctionType.Exp,
                    bias=esh[:kv_c], scale=SCALE,
                )
                prev = ((p16a, p16b), kc)
            while pend:
                pend.pop(0)()
            pending_tail[0] = mk_tail(s, q_s, poAll, emit_ot, prev)
        pending_tail[0]()


def _build_nc(slots):
    nc = bacc.Bacc(
        "TRN2",
        target_bir_lowering=False,
        debug=False,
        enable_asserts=False,
        num_devices=N_CORES,
    )
    dr = {}
    dr["apack"] = nc.dram_tensor("apack", [64, 5 * 128], F16,
                                 kind="ExternalInput").ap()
    dr["wvp"] = nc.dram_tensor("wvp", [64, VW], F16, kind="ExternalInput").ap()
    for s, (q_s, kv_s, nkc) in enumerate(slots):
        dr[f"qt{s}"] = nc.dram_tensor(f"qt{s}", [64, q_s], F16,
                                      kind="ExternalInput").ap()
        dr[f"kt{s}"] = nc.dram_tensor(f"kt{s}", [128, kv_s], F16,
                                      kind="ExternalInput").ap()
        dr[f"vt{s}"] = nc.dram_tensor(f"vt{s}", [64, kv_s], F16,
                                      kind="ExternalInput").ap()
        dr[f"ot{s}"] = nc.dram_tensor(f"ot{s}", [128, 5, q_s], F16,
                                      kind="ExternalOutput").ap()
    with tile.TileContext(nc) as tc:
        _emit(tc, nc, dr, slots)
    nc.compile()
    return nc


# ----------------------------------------------------------------- driver

def kernel(**inputs):
    global LAST_RESULT
    Q_seq = np.asarray(inputs["Q_seq"], dtype=np.float32)
    K_seq = np.asarray(inputs["K_seq"], dtype=np.float32)
    V_seq = np.asarray(inputs["V_seq"], dtype=np.float32)
    Q_len = np.asarray(inputs["Q_len"]).reshape(-1)
    V_len = np.asarray(inputs["V_len"]).reshape(-1)
    WQ_w = np.asarray(inputs["WQ_w"], dtype=np.float32)
    WQ_b = np.asarray(inputs["WQ_b"], dtype=np.float32)
    WK_w = np.asarray(inputs["WK_w"], dtype=np.float32)
    WK_b = np.asarray(inputs["WK_b"], dtype=np.float32)
    WV_w = np.asarray(inputs["WV_w"], dtype=np.float32)
    WV_b = np.asarray(inputs["WV_b"], dtype=np.float32)

    slots, grid = _plan(Q_len, V_len)
    nc = _build_nc(slots)

    apack = _pack_a(WQ_w, WQ_b, WK_w, WK_b)
    wvp = _pack_wv(WV_w, WV_b)

    in_maps = []
    for c in range(N_CORES):
        m = {"apack": apack, "wvp": wvp}
        for s, (q_s, kv_s, nkc) in enumerate(slots):
            u = grid[s][c]
            if u is None:
                m[f"qt{s}"] = np.zeros((64, q_s), np.float16)
                m[f"kt{s}"] = np.zeros((128, kv_s), np.float16)
                m[f"vt{s}"] = np.zeros((64, kv_s), np.float16)
            else:
                b, q0, q_e, kvlen = u
                m[f"qt{s}"] = _prep_rep(Q_seq[b, q0:q0 + q_e], q_s, q_e, 2)
                m[f"kt{s}"] = _prep_rep(K_seq[b], kv_s, kvlen, 4)
                m[f"vt{s}"] = _prep_rep(V_seq[b], kv_s, kvlen, 2)
        in_maps.append(m)

    res = run_bass_kernel_spmd(
        nc, in_maps, core_ids=list(range(N_CORES)), trace=TRACE
    )
    LAST_RESULT = res

    out = np.zeros((B, LQ, OUT_DIM), np.float32)
    for c in range(N_CORES):
        for s in range(len(slots)):
            u = grid[s][c]
            if u is None:
                continue
            b, q0, q_e, kvlen = u
            ot = np.asarray(res.results[c][f"ot{s}"], dtype=np.float32)
            out[b, q0:q0 + q_e] = unpack_ot(ot, q_e)
    return out


def unpack_ot(ot, q_e):
    """ot [128, 5, q_s] f32: row 32j+d = head 4g+j dim d (d=20 denom).
    Returns [q_e, 400]."""
    o4 = ot[:, :, :q_e].reshape(4, 32, 5, q_e)   # [j, d', g, q]
    num = o4[:, :20]                             # [j, d, g, q]
    den = o4[:, 20]                               # [j, g, q]
    val = num / den[:, None, :, :]
    return val.transpose(3, 2, 0, 1).reshape(q_e, 400)
